# revision 21
# baseline (speedup 1.0000x reference)
"""Single-head attention (B=8, N=2048, E=1024) on 8 TRN2 NeuronCores.

Sharding: data-parallel over batch — core i computes batch element i fully.
Host-side prep transposes x and W so the device kernel needs no transposes:
every matmul operand arrives with its contraction dim on SBUF partitions.

Per-core dataflow (all matmul compute in bf16, f32 PSUM accumulation):
  qT[f,n] = WT_lhsT.T @ xT_rhs      (projection, f on partitions)
  kT[f,n] = same
  v[n,e]  = xT_lhsT.T @ WTv_rhs     (natural layout, n on partitions)
  scoresT[j,i] = kT_lhsT.T @ qT_rhs ; expT = exp(scale*scoresT)  (ScalarE)
  denom[i] = ones-matmul over j-partitions of DVE-reduced exp sums
  out[i,e] = (expT_lhsT.T @ v_rhs) * (1/denom)
Softmax skips max-subtraction: scores are ~N(0,1) (max |s| < ~8), exp is
safe in f32 and softmax is shift-invariant.
"""

import numpy as np
import ml_dtypes

P = 128
E = 1024
N = 2048
F = 3072
KO = E // P      # 8 contraction subtiles
NT = N // P      # 16 row tiles
NCH = N // 512   # 4 chunks of 512
SCALE = 0.03125  # 1/sqrt(1024)

_CACHE = {}


def _build():
    import concourse.bacc as bacc
    import concourse.tile as tile
    import concourse.mybir as mybir

    f32 = mybir.dt.float32
    bf16 = mybir.dt.bfloat16
    AF = mybir.ActivationFunctionType
    Alu = mybir.AluOpType

    nc = bacc.Bacc("TRN2", target_bir_lowering=False, debug=False, num_devices=8)
    xT_d = nc.dram_tensor("xT", [E, N], bf16, kind="ExternalInput")
    WT_d = nc.dram_tensor("WT", [E, F], bf16, kind="ExternalInput")
    bqk_d = nc.dram_tensor("b_qk", [P, 16], f32, kind="ExternalInput")
    bv_d = nc.dram_tensor("b_v", [P, E], f32, kind="ExternalInput")
    out_d = nc.dram_tensor("out", [N, E], f32, kind="ExternalOutput")

    xT_r = xT_d.ap().rearrange("(ko p) n -> ko p n", p=P)
    WT_r = WT_d.ap().rearrange("(ko p) f -> ko p f", p=P)
    out_r = out_d.ap().rearrange("(it p) e -> it p e", p=P)

    with tile.TileContext(nc) as tc:
        with (
            tc.tile_pool(name="const", bufs=1) as const,
            tc.tile_pool(name="qkv", bufs=1) as qkv,
        ):
            bqk_t = const.tile([P, 16], f32, tag="bqk")
            nc.gpsimd.dma_start(bqk_t[:], bqk_d.ap())
            bv_t = const.tile([P, E], f32, tag="bv")
            nc.gpsimd.dma_start(bv_t[:], bv_d.ap())
            ones_t = const.tile([P, 1], f32, tag="ones")
            nc.vector.memset(ones_t[:], 1.0)

            # qT/kT split per n-chunk so attention chunk ic only depends on
            # the chunks it reads (finer scheduling deps than one big tile)
            qTc = [
                qkv.tile([P, KO, 512], bf16, tag=f"qT{c}", name=f"qT{c}")
                for c in range(NCH)
            ]
            kTc = [
                qkv.tile([P, KO, 512], bf16, tag=f"kT{c}", name=f"kT{c}")
                for c in range(NCH)
            ]
            vt = qkv.tile([P, NT, E], bf16, tag="v")

            with (
                tc.tile_pool(name="pin", bufs=1) as pin,
                tc.tile_pool(name="pproj", bufs=6, space="PSUM") as pproj,
            ):
                # Chunked input DMAs, issued in the order the projection
                # consumes them, so PE starts ~6us in instead of waiting for
                # the full 10MB load. xc[k][c]: x columns c*512..; wc[k][s]:
                # W columns s*512.. (s 0-1: q, 2-3: k, 4-5: v).
                xc = [[None] * NCH for _ in range(KO)]
                wc = [[None] * 6 for _ in range(KO)]

                def load_x(c):
                    for k in range(KO):
                        t = pin.tile([P, 512], bf16, tag=f"x{k}_{c}")
                        nc.sync.dma_start(t[:], xT_r[k][:, c * 512 : (c + 1) * 512])
                        xc[k][c] = t

                def load_w(s, eng):
                    # The first W section rides the ACT HWDGE ring (separate
                    # physical ring from SP) so it lands in parallel with x0;
                    # the rest queue on the SP ring in consumption order.
                    for k in range(KO):
                        t = pin.tile([P, 512], bf16, tag=f"w{k}_{s}")
                        eng.dma_start(t[:], WT_r[k][:, s * 512 : (s + 1) * 512])
                        wc[k][s] = t

                load_w(0, nc.scalar)
                load_x(0)
                load_w(1, nc.sync)
                load_w(2, nc.sync)
                load_w(3, nc.sync)
                load_x(1)
                load_w(4, nc.sync)
                load_w(5, nc.sync)
                load_x(2)
                load_x(3)

                # PE warmup: keep TensorE busy (and HAM at full clock) while
                # the first input chunks stream in. Results land in a junk
                # DRAM scratch so DCE keeps the chain.
                scratch = pin.tile([P, 512], bf16, tag="warm_in")
                nc.vector.memset(scratch[:], 0.0)
                junk_ps = pproj.tile([P, 512], f32, tag="warm_ps", bufs=1)
                for _ in range(28):
                    nc.tensor.matmul(
                        junk_ps[:], lhsT=scratch[:, :P], rhs=scratch[:],
                        start=True, stop=True,
                    )
                junk_sb = pin.tile([P, 1], f32, tag="warm_out")
                nc.vector.tensor_copy(junk_sb[:], junk_ps[:, 0:1])
                junk_d = nc.dram_tensor("warm_scratch", [P, 1], f32, kind="Internal")
                nc.sync.dma_start(junk_d.ap(), junk_sb[:])

                # q/k projection -> qT/kT [f(part), n], per-chunk tiles; the
                # last kT chunk's PSUM->SBUF copy drains during v-proj, so
                # attention never waits on it.
                for ch in range(NCH):
                    for ft in range(16):  # 0-7: q rows of W, 8-15: k rows
                        ps = pproj.tile([P, 512], f32, tag="ps")
                        for k in range(KO):
                            nc.tensor.matmul(
                                ps[:],
                                lhsT=wc[k][ft // 4][:, (ft % 4) * P : (ft % 4 + 1) * P],
                                rhs=xc[k][ch][:],
                                start=(k == 0),
                                stop=(k == KO - 1),
                            )
                        dst = (qTc if ft < 8 else kTc)[ch][:, ft % 8, :]
                        nc.scalar.activation(
                            dst, ps[:], AF.Identity, bias=bqk_t[:, ft : ft + 1], scale=1.0
                        )

                # v projection -> v [n(part), e]
                for nt in range(NT):
                    for ch2 in range(2):
                        esl = slice(ch2 * 512, (ch2 + 1) * 512)
                        ps = pproj.tile([P, 512], f32, tag="ps")
                        for k in range(KO):
                            nc.tensor.matmul(
                                ps[:],
                                lhsT=xc[k][nt // 4][:, (nt % 4) * P : (nt % 4 + 1) * P],
                                rhs=wc[k][4 + ch2][:],
                                start=(k == 0),
                                stop=(k == KO - 1),
                            )
                        nc.vector.tensor_tensor(
                            out=vt[:, nt, esl],
                            in0=ps[:],
                            in1=bv_t[:, esl],
                            op=Alu.add,
                        )

            with (
                tc.tile_pool(name="attn", bufs=2) as attn,
                tc.tile_pool(name="psc", bufs=2, space="PSUM") as psc,
                tc.tile_pool(name="pnum", bufs=4, space="PSUM") as pnum,
                tc.tile_pool(name="pden", bufs=2, space="PSUM") as pden,
            ):
                # Software pipeline: scores(ic) is emitted before the
                # denominator + numerator of (ic-1), so the DVE exp-sum
                # reduce of chunk ic-1 overlaps with scores matmuls of ic
                # instead of stalling PE.
                def emit_scores(ic):
                    expT = attn.tile([P, NT, 512], bf16, tag="expT", bufs=3)
                    for jt in range(NT):
                        ps = psc.tile([P, 512], f32, tag="ps_s")
                        for k in range(KO):
                            nc.tensor.matmul(
                                ps[:],
                                lhsT=kTc[jt // 4][:, k, (jt % 4) * P : (jt % 4 + 1) * P],
                                rhs=qTc[ic][:, k, :],
                                start=(k == 0),
                                stop=(k == KO - 1),
                            )
                        nc.scalar.activation(expT[:, jt, :], ps[:], AF.Exp, scale=SCALE)
                    # softmax denominators, step 1: sum over the 16 j-tiles
                    # (free-dim strided reduce on DVE)
                    sume = attn.tile([P, 512], f32, tag="sume")
                    nc.vector.reduce_sum(
                        sume[:],
                        expT.rearrange("p j i -> p i j"),
                        axis=mybir.AxisListType.X,
                    )
                    return expT, sume

                def emit_tail(ic, expT, sume):
                    for isub in range(4):
                        it = ic * 4 + isub
                        # step 2: sum over the remaining 128 j-partitions
                        psd = pden.tile([P, 1], f32, tag="ps_d")
                        nc.tensor.matmul(
                            psd[:],
                            lhsT=sume[:, isub * P : (isub + 1) * P],
                            rhs=ones_t[:],
                            start=True,
                            stop=True,
                        )
                        rden = attn.tile([P, 1], f32, tag="rden", bufs=4)
                        nc.vector.reciprocal(rden[:], psd[:])
                        osb = attn.tile([P, E], f32, tag="osb", bufs=3)
                        for ch2 in range(2):
                            esl = slice(ch2 * 512, (ch2 + 1) * 512)
                            ps = pnum.tile([P, 512], f32, tag="ps_n")
                            for jt in range(NT):
                                nc.tensor.matmul(
                                    ps[:],
                                    lhsT=expT[:, jt, isub * P : (isub + 1) * P],
                                    rhs=vt[:, jt, esl],
                                    start=(jt == 0),
                                    stop=(jt == NT - 1),
                                )
                            # division on ScalarE (Copy with per-partition
                            # scale) keeps the DVE free so the pden PSUM slot
                            # recycles without stalling the next denom matmul
                            nc.scalar.activation(
                                osb[:, esl], ps[:], AF.Copy, scale=rden[:]
                            )
                            nc.sync.dma_start(out_r[it][:, esl], osb[:, esl])

                prev = None
                for ic in range(NCH):
                    cur = emit_scores(ic)
                    if prev is not None:
                        emit_tail(ic - 1, *prev)
                    prev = cur
                emit_tail(NCH - 1, *prev)
    nc.compile()
    return nc


def get_nc():
    if "nc" not in _CACHE:
        _CACHE["nc"] = _build()
    return _CACHE["nc"]


def prepare_in_maps(x, W_qkv, b_qkv):
    bf = ml_dtypes.bfloat16
    x = np.asarray(x, dtype=np.float32)
    W = np.asarray(W_qkv, dtype=np.float32)
    b = np.asarray(b_qkv, dtype=np.float32)
    assert x.shape == (8, N, E) and W.shape == (F, E) and b.shape == (F,)
    xT = np.ascontiguousarray(np.transpose(x, (0, 2, 1))).astype(bf)  # [8, E, N]
    WT = np.ascontiguousarray(W.T).astype(bf)  # [E, F]
    bqk = np.ascontiguousarray(b[: 2 * E].reshape(16, P).T)  # [P, 16]
    bv = np.ascontiguousarray(np.broadcast_to(b[2 * E :], (P, E)))  # [P, E]
    return [{"xT": xT[i], "WT": WT, "b_qk": bqk, "b_v": bv} for i in range(8)]


def kernel(x, W_qkv, b_qkv):
    from concourse.bass_utils import run_bass_kernel_spmd

    nc = get_nc()
    in_maps = prepare_in_maps(x, W_qkv, b_qkv)
    res = run_bass_kernel_spmd(nc, in_maps, core_ids=list(range(8)))
    return np.stack([res.results[i]["out"] for i in range(8)], axis=0)


# revision 24
# speedup vs baseline: 1.1805x; 1.1805x over previous
"""Single-head attention (B=8, N=2048, E=1024) on 8 TRN2 NeuronCores.

Sharding: data-parallel over batch — core i computes batch element i fully.
Host-side prep transposes x and W so the device kernel needs no transposes:
every matmul operand arrives with its contraction dim on SBUF partitions.

Per-core dataflow (all matmul compute in bf16, f32 PSUM accumulation):
  qT[f,n] = WT_lhsT.T @ xT_rhs      (projection, f on partitions)
  kT[f,n] = same
  v[n,e]  = xT_lhsT.T @ WTv_rhs     (natural layout, n on partitions)
  scoresT[j,i] = kT_lhsT.T @ qT_rhs ; expT = exp(scale*scoresT)  (ScalarE)
  denom[i] = ones-matmul over j-partitions of DVE-reduced exp sums
  out[i,e] = (expT_lhsT.T @ v_rhs) * (1/denom)
Softmax skips max-subtraction: scores are ~N(0,1) (max |s| < ~8), exp is
safe in f32 and softmax is shift-invariant.
"""

import numpy as np
import ml_dtypes

P = 128
E = 1024
N = 2048
F = 3072
KO = E // P      # 8 contraction subtiles
NT = N // P      # 16 row tiles
NCH = N // 512   # 4 chunks of 512
SCALE = 0.03125  # 1/sqrt(1024)

_CACHE = {}


def _build():
    import concourse.bacc as bacc
    import concourse.tile as tile
    import concourse.mybir as mybir

    f32 = mybir.dt.float32
    bf16 = mybir.dt.bfloat16
    AF = mybir.ActivationFunctionType
    Alu = mybir.AluOpType

    nc = bacc.Bacc("TRN2", target_bir_lowering=False, debug=False, num_devices=8)
    xT_d = nc.dram_tensor("xT", [E, N], bf16, kind="ExternalInput")
    WT_d = nc.dram_tensor("WT", [E, F], bf16, kind="ExternalInput")
    bqk_d = nc.dram_tensor("b_qk", [P, 16], f32, kind="ExternalInput")
    bv_d = nc.dram_tensor("b_v", [P, E], f32, kind="ExternalInput")
    out_d = nc.dram_tensor("out", [N, E], f32, kind="ExternalOutput")

    xT_r = xT_d.ap().rearrange("(ko p) n -> ko p n", p=P)
    WT_r = WT_d.ap().rearrange("(ko p) f -> ko p f", p=P)
    out_r = out_d.ap().rearrange("(it p) e -> it p e", p=P)

    with tile.TileContext(nc) as tc:
        with (
            tc.tile_pool(name="const", bufs=1) as const,
            tc.tile_pool(name="qkv", bufs=1) as qkv,
        ):
            bqk_t = const.tile([P, 16], f32, tag="bqk")
            nc.gpsimd.dma_start(bqk_t[:], bqk_d.ap())
            bv_t = const.tile([P, E], f32, tag="bv")
            nc.gpsimd.dma_start(bv_t[:], bv_d.ap())
            ones_t = const.tile([P, 1], bf16, tag="ones")
            nc.vector.memset(ones_t[:], 1.0)

            # qT/kT split per n-chunk so attention chunk ic only depends on
            # the chunks it reads (finer scheduling deps than one big tile)
            qTc = [
                qkv.tile([P, KO, 512], bf16, tag=f"qT{c}", name=f"qT{c}")
                for c in range(NCH)
            ]
            kTc = [
                qkv.tile([P, KO, 512], bf16, tag=f"kT{c}", name=f"kT{c}")
                for c in range(NCH)
            ]
            vt = qkv.tile([P, NT, E], bf16, tag="v")

            with (
                tc.tile_pool(name="pin", bufs=1) as pin,
                tc.tile_pool(name="pproj", bufs=7, space="PSUM") as pproj,
            ):
                # Chunked input DMAs, issued in the order the projection
                # consumes them, so PE starts ~6us in instead of waiting for
                # the full 10MB load. xc[k][c]: x columns c*512..; wc[k][s]:
                # W columns s*512.. (s 0-1: q, 2-3: k, 4-5: v).
                xc = [[None] * NCH for _ in range(KO)]
                wc = [[None] * 6 for _ in range(KO)]

                def load_x(c):
                    for k in range(KO):
                        t = pin.tile([P, 512], bf16, tag=f"x{k}_{c}")
                        nc.sync.dma_start(t[:], xT_r[k][:, c * 512 : (c + 1) * 512])
                        xc[k][c] = t

                def load_w(s, eng):
                    # The first W section rides the ACT HWDGE ring (separate
                    # physical ring from SP) so it lands in parallel with x0;
                    # the rest queue on the SP ring in consumption order.
                    for k in range(KO):
                        t = pin.tile([P, 512], bf16, tag=f"w{k}_{s}")
                        eng.dma_start(t[:], WT_r[k][:, s * 512 : (s + 1) * 512])
                        wc[k][s] = t

                load_w(0, nc.scalar)
                load_x(0)
                load_w(1, nc.sync)
                load_w(2, nc.sync)
                load_w(3, nc.sync)
                load_x(1)
                load_w(4, nc.sync)
                load_w(5, nc.sync)
                load_x(2)
                load_x(3)

                # PE warmup: keep TensorE busy (and HAM at full clock) while
                # the first input chunks stream in. Results land in a junk
                # DRAM scratch so DCE keeps the chain.
                scratch = pin.tile([P, 512], bf16, tag="warm_in")
                nc.vector.memset(scratch[:], 0.0)
                junk_ps = pproj.tile([P, 512], f32, tag="warm_ps", bufs=1)
                for _ in range(28):
                    nc.tensor.matmul(
                        junk_ps[:], lhsT=scratch[:, :P], rhs=scratch[:],
                        start=True, stop=True,
                    )
                junk_sb = pin.tile([P, 1], f32, tag="warm_out")
                nc.vector.tensor_copy(junk_sb[:], junk_ps[:, 0:1])
                junk_d = nc.dram_tensor("warm_scratch", [P, 1], f32, kind="Internal")
                nc.sync.dma_start(junk_d.ap(), junk_sb[:])

                # q/k projection -> qT/kT [f(part), n], per-chunk tiles; the
                # last kT chunk's PSUM->SBUF copy drains during v-proj, so
                # attention never waits on it.
                for ch in range(NCH):
                    for ft in range(16):  # 0-7: q rows of W, 8-15: k rows
                        ps = pproj.tile([P, 512], f32, tag="ps")
                        for k in range(KO):
                            nc.tensor.matmul(
                                ps[:],
                                lhsT=wc[k][ft // 4][:, (ft % 4) * P : (ft % 4 + 1) * P],
                                rhs=xc[k][ch][:],
                                start=(k == 0),
                                stop=(k == KO - 1),
                            )
                        dst = (qTc if ft < 8 else kTc)[ch][:, ft % 8, :]
                        nc.scalar.activation(
                            dst, ps[:], AF.Identity, bias=bqk_t[:, ft : ft + 1], scale=1.0
                        )

                # v projection -> v [n(part), e]
                for nt in range(NT):
                    for ch2 in range(2):
                        esl = slice(ch2 * 512, (ch2 + 1) * 512)
                        ps = pproj.tile([P, 512], f32, tag="ps")
                        for k in range(KO):
                            nc.tensor.matmul(
                                ps[:],
                                lhsT=xc[k][nt // 4][:, (nt % 4) * P : (nt % 4 + 1) * P],
                                rhs=wc[k][4 + ch2][:],
                                start=(k == 0),
                                stop=(k == KO - 1),
                            )
                        nc.vector.tensor_tensor(
                            out=vt[:, nt, esl],
                            in0=ps[:],
                            in1=bv_t[:, esl],
                            op=Alu.add,
                        )

            with (
                tc.tile_pool(name="attn", bufs=2) as attn,
                tc.tile_pool(name="psc", bufs=2, space="PSUM") as psc,
                tc.tile_pool(name="pnum", bufs=4, space="PSUM") as pnum,
                tc.tile_pool(name="pden", bufs=2, space="PSUM") as pden,
            ):
                # Software pipeline: scores(ic) is emitted before the
                # denominator + numerator of (ic-1), so the DVE exp-sum
                # reduce of chunk ic-1 overlaps with scores matmuls of ic
                # instead of stalling PE.
                def emit_scores(ic):
                    expT = attn.tile([P, NT, 512], bf16, tag="expT", bufs=3)
                    for jt in range(NT):
                        ps = psc.tile([P, 512], f32, tag="ps_s")
                        for k in range(KO):
                            nc.tensor.matmul(
                                ps[:],
                                lhsT=kTc[jt // 4][:, k, (jt % 4) * P : (jt % 4 + 1) * P],
                                rhs=qTc[ic][:, k, :],
                                start=(k == 0),
                                stop=(k == KO - 1),
                            )
                        nc.scalar.activation(expT[:, jt, :], ps[:], AF.Exp, scale=SCALE)
                    # softmax denominators, step 1: sum over the 16 j-tiles
                    # (free-dim strided reduce on DVE)
                    sume = attn.tile([P, 512], f32, tag="sume")
                    nc.vector.reduce_sum(
                        sume[:],
                        expT.rearrange("p j i -> p i j"),
                        axis=mybir.AxisListType.X,
                    )
                    # bf16 copy so the cross-partition denominator matmul is a
                    # cheap bf16 op instead of a double-pass fp32 one
                    sume_bf = attn.tile([P, 512], bf16, tag="sume_bf")
                    nc.scalar.activation(sume_bf[:], sume[:], AF.Copy)
                    return expT, sume_bf

                def emit_tail(ic, expT, sume):
                    for isub in range(4):
                        it = ic * 4 + isub
                        # step 2: sum over the remaining 128 j-partitions
                        psd = pden.tile([P, 1], f32, tag="ps_d")
                        nc.tensor.matmul(
                            psd[:],
                            lhsT=sume[:, isub * P : (isub + 1) * P],
                            rhs=ones_t[:],
                            start=True,
                            stop=True,
                        )
                        rden = attn.tile([P, 1], f32, tag="rden", bufs=4)
                        nc.vector.reciprocal(rden[:], psd[:])
                        osb = attn.tile([P, E], f32, tag="osb", bufs=3)
                        for ch2 in range(2):
                            esl = slice(ch2 * 512, (ch2 + 1) * 512)
                            ps = pnum.tile([P, 512], f32, tag="ps_n")
                            for jt in range(NT):
                                nc.tensor.matmul(
                                    ps[:],
                                    lhsT=expT[:, jt, isub * P : (isub + 1) * P],
                                    rhs=vt[:, jt, esl],
                                    start=(jt == 0),
                                    stop=(jt == NT - 1),
                                )
                            # division on ScalarE (Copy with per-partition
                            # scale) keeps the DVE free so the pden PSUM slot
                            # recycles without stalling the next denom matmul
                            nc.scalar.activation(
                                osb[:, esl], ps[:], AF.Copy, scale=rden[:]
                            )
                            nc.sync.dma_start(out_r[it][:, esl], osb[:, esl])

                prev = None
                for ic in range(NCH):
                    cur = emit_scores(ic)
                    if prev is not None:
                        emit_tail(ic - 1, *prev)
                    prev = cur
                emit_tail(NCH - 1, *prev)
    nc.compile()
    return nc


def get_nc():
    if "nc" not in _CACHE:
        _CACHE["nc"] = _build()
    return _CACHE["nc"]


def prepare_in_maps(x, W_qkv, b_qkv):
    bf = ml_dtypes.bfloat16
    x = np.asarray(x, dtype=np.float32)
    W = np.asarray(W_qkv, dtype=np.float32)
    b = np.asarray(b_qkv, dtype=np.float32)
    assert x.shape == (8, N, E) and W.shape == (F, E) and b.shape == (F,)
    xT = np.ascontiguousarray(np.transpose(x, (0, 2, 1))).astype(bf)  # [8, E, N]
    WT = np.ascontiguousarray(W.T).astype(bf)  # [E, F]
    bqk = np.ascontiguousarray(b[: 2 * E].reshape(16, P).T)  # [P, 16]
    bv = np.ascontiguousarray(np.broadcast_to(b[2 * E :], (P, E)))  # [P, E]
    return [{"xT": xT[i], "WT": WT, "b_qk": bqk, "b_v": bv} for i in range(8)]


def kernel(x, W_qkv, b_qkv):
    from concourse.bass_utils import run_bass_kernel_spmd

    nc = get_nc()
    in_maps = prepare_in_maps(x, W_qkv, b_qkv)
    res = run_bass_kernel_spmd(nc, in_maps, core_ids=list(range(8)))
    return np.stack([res.results[i]["out"] for i in range(8)], axis=0)


# revision 25
# speedup vs baseline: 1.2071x; 1.0225x over previous
"""Single-head attention (B=8, N=2048, E=1024) on 8 TRN2 NeuronCores.

Sharding: data-parallel over batch — core i computes batch element i fully.
Host-side prep transposes x and W so the device kernel needs no transposes:
every matmul operand arrives with its contraction dim on SBUF partitions.

Per-core dataflow (all matmul compute in bf16, f32 PSUM accumulation):
  qT[f,n] = WT_lhsT.T @ xT_rhs      (projection, f on partitions)
  kT[f,n] = same
  v[n,e]  = xT_lhsT.T @ WTv_rhs     (natural layout, n on partitions)
  scoresT[j,i] = kT_lhsT.T @ qT_rhs ; expT = exp(scale*scoresT)  (ScalarE)
  denom[i] = ones-matmul over j-partitions of DVE-reduced exp sums
  out[i,e] = (expT_lhsT.T @ v_rhs) * (1/denom)
Softmax skips max-subtraction: scores are ~N(0,1) (max |s| < ~8), exp is
safe in f32 and softmax is shift-invariant.
"""

import numpy as np
import ml_dtypes

P = 128
E = 1024
N = 2048
F = 3072
KO = E // P      # 8 contraction subtiles
NT = N // P      # 16 row tiles
NCH = N // 512   # 4 chunks of 512
SCALE = 0.03125  # 1/sqrt(1024)

_CACHE = {}


def _build():
    import concourse.bacc as bacc
    import concourse.tile as tile
    import concourse.mybir as mybir

    f32 = mybir.dt.float32
    bf16 = mybir.dt.bfloat16
    AF = mybir.ActivationFunctionType
    Alu = mybir.AluOpType

    nc = bacc.Bacc("TRN2", target_bir_lowering=False, debug=False, num_devices=8)
    xT_d = nc.dram_tensor("xT", [E, N], bf16, kind="ExternalInput")
    WT_d = nc.dram_tensor("WT", [E, F], bf16, kind="ExternalInput")
    bqk_d = nc.dram_tensor("b_qk", [P, 16], f32, kind="ExternalInput")
    bv_d = nc.dram_tensor("b_v", [P, E], f32, kind="ExternalInput")
    out_d = nc.dram_tensor("out", [N, E], f32, kind="ExternalOutput")

    xT_r = xT_d.ap().rearrange("(ko p) n -> ko p n", p=P)
    WT_r = WT_d.ap().rearrange("(ko p) f -> ko p f", p=P)
    out_r = out_d.ap().rearrange("(it p) e -> it p e", p=P)

    with tile.TileContext(nc) as tc:
        with (
            tc.tile_pool(name="const", bufs=1) as const,
            tc.tile_pool(name="qkv", bufs=1) as qkv,
        ):
            bqk_t = const.tile([P, 16], f32, tag="bqk")
            nc.gpsimd.dma_start(bqk_t[:], bqk_d.ap())
            bv_t = const.tile([P, E], f32, tag="bv")
            nc.gpsimd.dma_start(bv_t[:], bv_d.ap())
            ones_t = const.tile([P, 1], bf16, tag="ones")
            nc.vector.memset(ones_t[:], 1.0)

            # qT/kT split per n-chunk so attention chunk ic only depends on
            # the chunks it reads (finer scheduling deps than one big tile)
            qTc = [
                qkv.tile([P, KO, 512], bf16, tag=f"qT{c}", name=f"qT{c}")
                for c in range(NCH)
            ]
            kTc = [
                qkv.tile([P, KO, 512], bf16, tag=f"kT{c}", name=f"kT{c}")
                for c in range(NCH)
            ]
            vt = qkv.tile([P, NT, E], bf16, tag="v")

            with (
                tc.tile_pool(name="pin", bufs=1) as pin,
                tc.tile_pool(name="pproj", bufs=7, space="PSUM") as pproj,
            ):
                # Chunked input DMAs, issued in the order the projection
                # consumes them, so PE starts ~6us in instead of waiting for
                # the full 10MB load. xc[k][c]: x columns c*512..; wc[k][s]:
                # W columns s*512.. (s 0-1: q, 2-3: k, 4-5: v).
                xc = [[None] * NCH for _ in range(KO)]
                wc = [[None] * 6 for _ in range(KO)]

                def load_x(c):
                    for k in range(KO):
                        t = pin.tile([P, 512], bf16, tag=f"x{k}_{c}")
                        nc.sync.dma_start(t[:], xT_r[k][:, c * 512 : (c + 1) * 512])
                        xc[k][c] = t

                def load_w(s, eng):
                    # The first W section rides the ACT HWDGE ring (separate
                    # physical ring from SP) so it lands in parallel with x0;
                    # the rest queue on the SP ring in consumption order.
                    for k in range(KO):
                        t = pin.tile([P, 512], bf16, tag=f"w{k}_{s}")
                        eng.dma_start(t[:], WT_r[k][:, s * 512 : (s + 1) * 512])
                        wc[k][s] = t

                load_w(0, nc.scalar)
                load_x(0)
                load_w(1, nc.sync)
                load_w(2, nc.sync)
                load_w(3, nc.sync)
                load_x(1)
                load_w(4, nc.sync)
                load_w(5, nc.sync)
                load_x(2)
                load_x(3)

                # PE warmup: keep TensorE busy (and HAM at full clock) while
                # the first input chunks stream in. Results land in a junk
                # DRAM scratch so DCE keeps the chain.
                scratch = pin.tile([P, 512], bf16, tag="warm_in")
                nc.vector.memset(scratch[:], 0.0)
                junk_ps = pproj.tile([P, 512], f32, tag="warm_ps", bufs=1)
                for _ in range(28):
                    nc.tensor.matmul(
                        junk_ps[:], lhsT=scratch[:, :P], rhs=scratch[:],
                        start=True, stop=True,
                    )
                junk_sb = pin.tile([P, 1], f32, tag="warm_out")
                nc.vector.tensor_copy(junk_sb[:], junk_ps[:, 0:1])
                junk_d = nc.dram_tensor("warm_scratch", [P, 1], f32, kind="Internal")
                nc.sync.dma_start(junk_d.ap(), junk_sb[:])

                # q/k projection -> qT/kT [f(part), n], per-chunk tiles; the
                # last kT chunk's PSUM->SBUF copy drains during v-proj, so
                # attention never waits on it.
                for ch in range(NCH):
                    for ft in range(16):  # 0-7: q rows of W, 8-15: k rows
                        ps = pproj.tile([P, 512], f32, tag="ps")
                        for k in range(KO):
                            nc.tensor.matmul(
                                ps[:],
                                lhsT=wc[k][ft // 4][:, (ft % 4) * P : (ft % 4 + 1) * P],
                                rhs=xc[k][ch][:],
                                start=(k == 0),
                                stop=(k == KO - 1),
                            )
                        dst = (qTc if ft < 8 else kTc)[ch][:, ft % 8, :]
                        nc.scalar.activation(
                            dst, ps[:], AF.Identity, bias=bqk_t[:, ft : ft + 1], scale=1.0
                        )

                # v projection -> v [n(part), e]
                for nt in range(NT):
                    for ch2 in range(2):
                        esl = slice(ch2 * 512, (ch2 + 1) * 512)
                        ps = pproj.tile([P, 512], f32, tag="ps")
                        for k in range(KO):
                            nc.tensor.matmul(
                                ps[:],
                                lhsT=xc[k][nt // 4][:, (nt % 4) * P : (nt % 4 + 1) * P],
                                rhs=wc[k][4 + ch2][:],
                                start=(k == 0),
                                stop=(k == KO - 1),
                            )
                        nc.vector.tensor_tensor(
                            out=vt[:, nt, esl],
                            in0=ps[:],
                            in1=bv_t[:, esl],
                            op=Alu.add,
                        )

            with (
                tc.tile_pool(name="attn", bufs=2) as attn,
                tc.tile_pool(name="psc", bufs=2, space="PSUM") as psc,
                tc.tile_pool(name="pnum", bufs=4, space="PSUM") as pnum,
                tc.tile_pool(name="pden", bufs=2, space="PSUM") as pden,
            ):
                # Software pipeline: scores(ic) is emitted before the
                # denominator + numerator of (ic-1), so the DVE exp-sum
                # reduce of chunk ic-1 overlaps with scores matmuls of ic
                # instead of stalling PE.
                def emit_scores(ic):
                    expT = attn.tile([P, NT, 512], bf16, tag="expT", bufs=3)
                    for jt in range(NT):
                        ps = psc.tile([P, 512], f32, tag="ps_s")
                        for k in range(KO):
                            nc.tensor.matmul(
                                ps[:],
                                lhsT=kTc[jt // 4][:, k, (jt % 4) * P : (jt % 4 + 1) * P],
                                rhs=qTc[ic][:, k, :],
                                start=(k == 0),
                                stop=(k == KO - 1),
                            )
                        nc.scalar.activation(expT[:, jt, :], ps[:], AF.Exp, scale=SCALE)
                    # softmax denominators, step 1: sum over the 16 j-tiles
                    # (free-dim strided reduce on DVE)
                    sume = attn.tile([P, 512], f32, tag="sume")
                    nc.vector.reduce_sum(
                        sume[:],
                        expT.rearrange("p j i -> p i j"),
                        axis=mybir.AxisListType.X,
                    )
                    # bf16 copy so the cross-partition denominator matmul is a
                    # cheap bf16 op instead of a double-pass fp32 one. On DVE
                    # (not ACT): it waits on the reduce, and ACT's FIFO must
                    # stay clear for the next chunk's EXPs.
                    sume_bf = attn.tile([P, 512], bf16, tag="sume_bf")
                    nc.vector.tensor_copy(sume_bf[:], sume[:])
                    return expT, sume_bf

                def emit_tail(ic, expT, sume):
                    for isub in range(4):
                        it = ic * 4 + isub
                        # step 2: sum over the remaining 128 j-partitions
                        psd = pden.tile([P, 1], f32, tag="ps_d")
                        nc.tensor.matmul(
                            psd[:],
                            lhsT=sume[:, isub * P : (isub + 1) * P],
                            rhs=ones_t[:],
                            start=True,
                            stop=True,
                        )
                        rden = attn.tile([P, 1], f32, tag="rden", bufs=4)
                        nc.vector.reciprocal(rden[:], psd[:])
                        osb = attn.tile([P, E], f32, tag="osb", bufs=3)
                        for ch2 in range(2):
                            esl = slice(ch2 * 512, (ch2 + 1) * 512)
                            ps = pnum.tile([P, 512], f32, tag="ps_n")
                            for jt in range(NT):
                                nc.tensor.matmul(
                                    ps[:],
                                    lhsT=expT[:, jt, isub * P : (isub + 1) * P],
                                    rhs=vt[:, jt, esl],
                                    start=(jt == 0),
                                    stop=(jt == NT - 1),
                                )
                            # division on ScalarE (Copy with per-partition
                            # scale) keeps the DVE free so the pden PSUM slot
                            # recycles without stalling the next denom matmul
                            nc.scalar.activation(
                                osb[:, esl], ps[:], AF.Copy, scale=rden[:]
                            )
                            nc.sync.dma_start(out_r[it][:, esl], osb[:, esl])

                prev = None
                for ic in range(NCH):
                    cur = emit_scores(ic)
                    if prev is not None:
                        emit_tail(ic - 1, *prev)
                    prev = cur
                emit_tail(NCH - 1, *prev)
    nc.compile()
    return nc


def get_nc():
    if "nc" not in _CACHE:
        _CACHE["nc"] = _build()
    return _CACHE["nc"]


def prepare_in_maps(x, W_qkv, b_qkv):
    bf = ml_dtypes.bfloat16
    x = np.asarray(x, dtype=np.float32)
    W = np.asarray(W_qkv, dtype=np.float32)
    b = np.asarray(b_qkv, dtype=np.float32)
    assert x.shape == (8, N, E) and W.shape == (F, E) and b.shape == (F,)
    xT = np.ascontiguousarray(np.transpose(x, (0, 2, 1))).astype(bf)  # [8, E, N]
    WT = np.ascontiguousarray(W.T).astype(bf)  # [E, F]
    bqk = np.ascontiguousarray(b[: 2 * E].reshape(16, P).T)  # [P, 16]
    bv = np.ascontiguousarray(np.broadcast_to(b[2 * E :], (P, E)))  # [P, E]
    return [{"xT": xT[i], "WT": WT, "b_qk": bqk, "b_v": bv} for i in range(8)]


def kernel(x, W_qkv, b_qkv):
    from concourse.bass_utils import run_bass_kernel_spmd

    nc = get_nc()
    in_maps = prepare_in_maps(x, W_qkv, b_qkv)
    res = run_bass_kernel_spmd(nc, in_maps, core_ids=list(range(8)))
    return np.stack([res.results[i]["out"] for i in range(8)], axis=0)
